# revision 72
# baseline (speedup 1.0000x reference)
"""Trainium2 Bass kernel for nn_AdaptiveNoiseScheduler (segment_reduce).

Distribution: 8 NeuronCores = 4 batches x 2 sequence-halves, T=2048 tokens per
core, MLP weights replicated. The fwd/bwd context tensor is computed ON THE PE
as per-128-token-block matmuls: for block b,

    ctx[t] = A_t*hsum + (B_t-A_t)*hpre_b + sum_{s in block} M_b[t,s]*h[s]
    M_b[t,s] = (B_t-A_t) for s<t, -A_t for s=t, 0 for s>t

with A_t = 0.5/(S-1-t) (gated), B_t = 0.5/t (gated). One fp8 DoubleRow matmul
per (block, e-subblock): stationary group0 = the h block (natural token-major
layout), group1 = {hsum, hpre_b} rows in "double-fp8" (value + residual) with
zeros below; moving group0 = 64*M_b^T fp8, group1 = the 64*A/64*(B-A)
coefficient rows. hsum/hpre (cheap O(B*S*E) numpy sums) come from the host.
PSUM accumulates the whole context in f32, one Copy-activation casts it to fp8
ctx^T tiles, and the rest is the reference MLP as plain fp8 DoubleRow GEMMs
(host-prescaled weights, gelu via the scalar engine's scale/bias ports). No
scans, no transposes, no cross-core collectives.
"""

from contextlib import ExitStack

import numpy as np
import ml_dtypes

P = 128
B, S, E = 4, 4096, 1024
T = S // 2              # tokens per core
NB = T // P             # 16 token blocks per core
NC = 4                  # 512-token L1/L2/L3 chunks per core
CH = T // NC            # 512
F1, F2 = 1024, 512
NE, NF1, NF2 = E // P, F1 // P, F2 // P
NUM_TIMESTEPS = 1000
FP8_MAX = 240.0  # TRN float8e4 is IEEE e4m3 (inf/nan present): max finite 240

_COMPILED = None


def _build_nc(act="Gelu"):
    import concourse.mybir as mybir
    import concourse.tile as tile
    from concourse import bacc

    f32 = mybir.dt.float32
    fp8 = mybir.dt.float8e4
    AF = mybir.ActivationFunctionType
    ACT_FN = getattr(AF, act)
    DR = mybir.MatmulPerfMode.DoubleRow

    nc = bacc.Bacc("TRN2", target_bir_lowering=False, debug=False, num_devices=8)

    hN_d = nc.dram_tensor("hN", (P, NB, NE, P), fp8, kind="ExternalInput").ap()
    # 4-partition sliver {hsum, hsum_res, hpre_b, hpre_res} for hN group1
    aug_d = nc.dram_tensor("aug4", (4, NB, NE, P), fp8, kind="ExternalInput").ap()
    mvt_d = nc.dram_tensor("mvt", (P, NB, 2, P), fp8, kind="ExternalInput").ap()
    hT_d = nc.dram_tensor("hT", (P, NE, T), fp8, kind="ExternalInput").ap()
    w1_d = nc.dram_tensor("w1pk", (P, NF1, 8, 2, P), fp8, kind="ExternalInput").ap()
    w2_d = nc.dram_tensor("w2pk", (P, NF2, 4, 2, P), fp8, kind="ExternalInput").ap()
    w3_d = nc.dram_tensor("w3u", (P, 2, 2, 1), fp8, kind="ExternalInput").ap()
    bcs_d = nc.dram_tensor("bcs", (P, NF1 + NF2), f32, kind="ExternalInput").ap()
    # logits token-on-partition: out[p, b] = 32*logit(token b*128+p)
    out_d = nc.dram_tensor("out", (P, NB), f32, kind="ExternalOutput").ap()

    with tile.TileContext(nc) as tc, ExitStack() as ctx:
        const = ctx.enter_context(tc.tile_pool(name="const", bufs=1))
        big = ctx.enter_context(tc.tile_pool(name="big", bufs=1))
        psC = ctx.enter_context(tc.tile_pool(name="psC", bufs=4, space="PSUM"))
        psM = ctx.enter_context(tc.tile_pool(name="psM", bufs=3, space="PSUM"))
        psN = ctx.enter_context(tc.tile_pool(name="psN", bufs=1, space="PSUM"))

        # ---- SBUF tiles
        # hN group0 = the h block (token-partition); group1 = the aug rows
        # {hsum, hsum_res, hpre_b, hpre_res} on partitions 0-3 (memset + DMA
        # sliver), so each ctx matmul is ONE start+stop instruction — PSUM
        # zero-region semantics forbid concurrent multi-instruction groups
        # in one 2KB zone, and the scheduler interleaves column chains.
        hN_t = big.tile([P, NB, 2, NE, P], fp8, name="hN_t")
        mvt_t = const.tile([P, NB, 2, P], fp8, name="mvt_t")
        hT_t = big.tile([P, NE, T], fp8, name="hT_t")
        ctxT = big.tile([P, NE, T], fp8, name="ctxT")
        x1 = big.tile([P, NF1, T], fp8, name="x1")
        x2 = big.tile([P, NF2, T], fp8, name="x2")
        nf2 = big.tile([P, NB], f32, name="nf2")
        w1_t = big.tile([P, NF1, 8, 2, P], fp8, name="w1_t")
        w2_t = const.tile([P, NF2, 4, 2, P], fp8, name="w2_t")
        w3_t = const.tile([P, 2, 2, 1], fp8, name="w3_t")
        bcs = const.tile([P, NF1 + NF2], f32, name="bcs")

        # ---- DMA prologue. One HWDGE ring (SP): issue (625ns) and the
        # DMA-engine pool are both serial global resources in the cost model,
        # so what matters is copy COUNT and exact need-order of the byte
        # stream. Pool zeroes hN group1 ahead of the 4-partition aug slivers;
        # only the tiny w3 rides the Pool DGE.
        wq = nc.gpsimd

        def load_hN(b0, n):
            # group1: memset then the aug sliver, both on the Pool queue so
            # the SP ring keeps its 625ns issue slots for the big copies
            nc.gpsimd.memset(hN_t[:, b0:b0 + n, 1], 0.0)
            wq.dma_start(hN_t[0:4, b0:b0 + n, 1], aug_d[:, b0:b0 + n])
            nc.sync.dma_start(hN_t[:, b0:b0 + n, 0], hN_d[:, b0:b0 + n])

        def load_w1(f0, n):  # n f-blocks, all 8 contraction layers: n*128KB
            nc.sync.dma_start(w1_t[:, f0:f0 + n], w1_d[:, f0:f0 + n])

        def load_hT(c):
            nc.sync.dma_start(
                hT_t[:, :, c * CH:(c + 1) * CH], hT_d[:, :, c * CH:(c + 1) * CH]
            )

        nc.sync.dma_start(mvt_t[:, 0:2], mvt_d[:, 0:2])
        load_hN(0, 1)
        load_hN(1, 2)
        load_hT(0)
        load_w1(0, 2)
        nc.sync.dma_start(bcs[:], bcs_d[:])
        load_hN(3, 3)
        load_w1(2, 2)
        nc.sync.dma_start(mvt_t[:, 2:8], mvt_d[:, 2:8])
        load_w1(4, 2)
        load_hN(6, 3)
        load_w1(6, 2)
        load_hT(1)
        nc.sync.dma_start(mvt_t[:, 8:], mvt_d[:, 8:])
        load_hN(9, 4)
        load_hT(2)
        nc.sync.dma_start(w2_t[:], w2_d[:])
        load_hT(3)
        load_hN(13, 3)
        wq.dma_start(w3_t[:], w3_d[:])

        # ---- context block: ONE start+stop DR matmul per e-sub (group0 =
        # h block x M^T, group1 = aug rows x coefficient rows), then a DVE
        # fp8 cast of the f32 PSUM accumulation per half block.
        def ctx_block(b):
            for h2 in range(2):
                psc = psC.tile([P, 4 * P], f32, tag="psC", name=f"psc_{b}_{h2}")
                for ai in range(4):
                    a = 4 * h2 + ai
                    nc.tensor.matmul(
                        psc[:, ai * P:(ai + 1) * P],
                        hN_t[:, b, :, a, :],
                        mvt_t[:, b, :, :],
                        start=True, stop=True,
                        perf_mode=DR,
                    )
                dst = ctxT[:, 4 * h2:4 * h2 + 4, b * P:(b + 1) * P]
                src = psc[:].rearrange("p (a t) -> p a t", a=4)
                # odd blocks put one half on ACT so the two copies overlap
                # (the Copy/Gelu act-table set loads only twice, not per switch)
                if b % 2 == 1 and h2 == 1:
                    nc.scalar.mul(dst, src, 1.0 / 16.0)
                else:
                    nc.vector.tensor_scalar_mul(dst, src, 1.0 / 16.0)

        ctx_iter = iter(range(NB))

        def emit_ctx(n):
            for _ in range(n):
                b = next(ctx_iter, None)
                if b is not None:
                    ctx_block(b)

        # ---- L1 (pre1 = [h, ctx] @ [W1a*32; W1b*8], gelu -> x1 fp8)
        def l1_pair(t0, tw, fg):
            tsl = slice(t0, t0 + tw)
            fs = (2 * fg, 2 * fg + 1)
            ps = [psM.tile([P, tw], f32, tag="psM", name=f"psP_{t0}_{f}") for f in fs]
            for e2 in range(8):
                if e2 < 4:
                    mov = hT_t[:, 2 * e2:2 * e2 + 2, tsl]
                else:
                    mov = ctxT[:, 2 * (e2 - 4):2 * (e2 - 4) + 2, tsl]
                for i, f in enumerate(fs):
                    nc.tensor.matmul(
                        ps[i], w1_t[:, f, e2], mov,
                        start=(e2 == 0), stop=(e2 == 7),
                        perf_mode=DR,
                    )
            for i, f in enumerate(fs):
                nc.scalar.activation(
                    x1[:, f, tsl], ps[i][:], ACT_FN,
                    bias=bcs[:, f:f + 1], scale=float(1.0 / 32.0),
                )

        def l2_one(t0, tw, f2):
            tsl = slice(t0, t0 + tw)
            psx = psM.tile([P, tw], f32, tag="psM", name=f"psX_{t0}_{f2}")
            for r in range(4):
                nc.tensor.matmul(
                    psx, w2_t[:, f2, r],
                    x1[:, 2 * r:2 * r + 2, tsl],
                    start=(r == 0), stop=(r == 3),
                    perf_mode=DR,
                )
            nc.scalar.activation(
                x2[:, f2, tsl], psx[:], ACT_FN,
                bias=bcs[:, NF1 + f2:NF1 + f2 + 1], scale=float(1.0 / 32.0),
            )

        # L3 with x2 as the DR stationary (dual-fp8 Ldweights forbids the
        # 1-wide w3 stationary) and w3 moving: one PSUM logit column per
        # (128-token block, r3 half) — every matmul is its own start+stop
        # group (zone-interleaving safe) — summed on the DVE.
        psn_t = psN.tile([P, 2, NB], f32, tag="psN", name="psn_t")

        def l3_one(t0, tw):
            c0 = t0 // P
            for tb in range(tw // P):
                for r3 in range(2):
                    nc.tensor.matmul(
                        psn_t[:, r3, c0 + tb:c0 + tb + 1],
                        x2[:, 2 * r3:2 * r3 + 2, t0 + tb * P:t0 + (tb + 1) * P],
                        w3_t[:, r3],
                        start=True, stop=True,
                        perf_mode=DR,
                    )
            # DVE may read at most one PSUM operand per instruction
            nc.vector.tensor_copy(nf2[:, c0:c0 + tw // P], psn_t[:, 0, c0:c0 + tw // P])
            nc.vector.tensor_add(
                nf2[:, c0:c0 + tw // P],
                nf2[:, c0:c0 + tw // P],
                psn_t[:, 1, c0:c0 + tw // P],
            )

        # ---- schedule: warm the PE with early ctx blocks, then pipeline
        # chunks; sprinkle remaining ctx blocks between units so the PE never
        # waits on a long ACT copy chain.
        emit_ctx(4)  # chunk 0's ctx layers read ctxT blocks 0-3
        chunks = [(0, 512), (512, 512), (1024, 512), (1536, 512)]
        for ci, (t0, tw) in enumerate(chunks):
            for fg in range(4):
                l1_pair(t0, tw, fg)
                emit_ctx(1)
            last = ci == len(chunks) - 1
            # split the last chunk's L2/L3 so the final gelu->L3->out cascade
            # drains over 256 tokens instead of 512
            for u0, uw in ([(t0, 256), (t0 + 256, 256)] if last else [(t0, tw)]):
                for f2 in range(NF2):
                    l2_one(u0, uw, f2)
                emit_ctx(1)
                l3_one(u0, uw)
        emit_ctx(NB)  # safety: flush any not yet emitted
        nc.sync.dma_start(out_d[:], nf2[:])

    nc.compile()
    return nc


def _get_compiled():
    global _COMPILED
    if _COMPILED is None:
        _COMPILED = _build_nc()
    return _COMPILED


def _f8(x):
    return np.clip(np.asarray(x, dtype=np.float32), -FP8_MAX, FP8_MAX).astype(
        ml_dtypes.float8_e4m3
    )


def _make_in_maps(inputs):
    h = np.ascontiguousarray(np.asarray(inputs["hidden_states"], dtype=np.float32))
    W1 = np.asarray(inputs["W1"], dtype=np.float32)
    W2 = np.asarray(inputs["W2"], dtype=np.float32)
    W3 = np.asarray(inputs["W3"], dtype=np.float32)
    b1 = np.asarray(inputs["b1"], dtype=np.float32)
    b2 = np.asarray(inputs["b2"], dtype=np.float32)

    i = np.arange(S, dtype=np.float64)
    A = np.where(i < S - 1, 0.5 / np.maximum(S - 1 - i, 1), 0.0).astype(np.float32)
    Bv = np.where(i > 0, 0.5 / np.maximum(i, 1), 0.0).astype(np.float32)
    BA = (Bv - A).astype(np.float32)

    # packed exactly in SBUF tile layout: w1pk[p, f, e2, g, x] = W1s[e2*256+g*128+p, f*128+x]
    w1s = _f8(np.vstack([W1[:E] * np.float32(32.0), W1[E:] * np.float32(8.0)]))
    w1_f8 = np.ascontiguousarray(
        w1s.reshape(8, 2, P, NF1, P).transpose(2, 3, 0, 1, 4)
    )
    w2_f8 = np.ascontiguousarray(
        _f8(W2 * np.float32(32.0)).reshape(4, 2, P, NF2, P).transpose(2, 3, 0, 1, 4)
    )
    # w3u[p, r3, g, 0] = 32*W3[r3*256 + g*128 + p]
    w3u = _f8((W3[:, 0] * np.float32(32.0)).reshape(2, 2, P).transpose(2, 0, 1)[..., None])
    b1c = np.ascontiguousarray(b1.reshape(NF1, P).T)
    b2c = np.ascontiguousarray(b2.reshape(NF2, P).T)

    sloc = np.arange(P)
    lower = sloc[:, None] < sloc[None, :]          # strictly below diagonal (s < t')
    eye = sloc[:, None] == sloc[None, :]

    in_maps = []
    for core in range(8):
        bi, half = divmod(core, 2)
        tg0 = half * T
        hs = h[bi, tg0:tg0 + T]                     # (T, E)
        Al = A[tg0:tg0 + T]
        BAl = BA[tg0:tg0 + T]

        hsum = h[bi].sum(0, dtype=np.float64).astype(np.float32)
        csum = np.concatenate(
            [np.zeros((1, E), np.float64), np.cumsum(h[bi], 0, dtype=np.float64)]
        )
        s1 = _f8(hsum)
        s2 = _f8(hsum - s1.astype(np.float32))

        # hN group0: [s, b, a, e'] = the h blocks. aug4: the 4-partition
        # group1 sliver {s1, s2, q1_b, q2_b} (device memsets the rest of
        # group1 to zero).
        hN = _f8(hs).reshape(NB, P, NE, P).transpose(1, 0, 2, 3)
        aug4 = np.zeros((4, NB, NE, P), dtype=ml_dtypes.float8_e4m3)
        aug4[0] = np.broadcast_to(s1.reshape(NE, P), (NB, NE, P))
        aug4[1] = np.broadcast_to(s2.reshape(NE, P), (NB, NE, P))
        hpre = csum[tg0 + np.arange(NB) * P].astype(np.float32)   # (NB, E)
        q1 = _f8(hpre)
        aug4[2] = q1.reshape(NB, NE, P)
        aug4[3] = _f8(hpre - q1.astype(np.float32)).reshape(NB, NE, P)

        # mvt: [s, b, g, t']  g0 = 64*M_b^T, g1 = coef rows 64*{A, A, BA, BA}
        # on partitions 0-3 (pairs the aug rows in hN group1), zeros below.
        Ab = Al.reshape(NB, P)
        BAb = BAl.reshape(NB, P)
        mvt = np.zeros((P, NB, 2, P), dtype=ml_dtypes.float8_e4m3)
        m = (
            lower[None, :, :] * (64.0 * BAb)[:, None, :]
            - eye[None, :, :] * (64.0 * Ab)[:, None, :]
        )                                            # (NB, s, t')
        mvt[:, :, 0] = _f8(m).transpose(1, 0, 2)
        mvt[0, :, 1] = _f8(64.0 * Ab)
        mvt[1, :, 1] = _f8(64.0 * Ab)
        mvt[2, :, 1] = _f8(64.0 * BAb)
        mvt[3, :, 1] = _f8(64.0 * BAb)

        hT = _f8(hs.T).reshape(NE, P, T).transpose(1, 0, 2)

        in_maps.append({
            "hN": np.ascontiguousarray(hN),
            "aug4": np.ascontiguousarray(aug4),
            "mvt": np.ascontiguousarray(mvt),
            "hT": np.ascontiguousarray(hT),
            "w1pk": w1_f8,
            "w2pk": w2_f8,
            "w3u": np.ascontiguousarray(w3u),
            "bcs": np.ascontiguousarray(np.hstack([b1c, b2c])),
        })
    return in_maps


def _finish(logits, inputs):
    b3 = np.asarray(inputs["b3"], dtype=np.float32)
    nfv = np.float32(1.0) / (np.float32(1.0) + np.exp(-(logits / np.float32(32.0) + b3[0])))
    gt = np.float32(np.asarray(inputs["global_timestep"]))
    mask = np.asarray(inputs["token_mask"])
    ad = gt * (np.float32(0.5) + nfv.astype(np.float32))
    ad = ad * (np.float32(1.0) + mask.astype(np.float32) * np.float32(0.3))
    ad = np.clip(ad, np.float32(0.0), np.float32(NUM_TIMESTEPS - 1))
    return ad.astype(np.int32)


def kernel(**inputs):
    from concourse import bass_utils

    nc = _get_compiled()
    in_maps = _make_in_maps(inputs)
    res = bass_utils.run_bass_kernel_spmd(nc, in_maps, core_ids=list(range(8)))
    logits = np.zeros((B, S), np.float32)
    for c in range(8):
        bi, half = divmod(c, 2)
        r = np.asarray(res.results[c]["out"])          # [P, NB], token b*128+p
        logits[bi, half * T:(half + 1) * T] = r.T.reshape(T)
    return _finish(logits, inputs)


# revision 73
# speedup vs baseline: 1.1518x; 1.1518x over previous
"""Trainium2 Bass kernel for nn_AdaptiveNoiseScheduler (segment_reduce).

Distribution: 8 NeuronCores = 4 batches x 2 sequence-halves, T=2048 tokens per
core, MLP weights replicated. The fwd/bwd context tensor is computed ON THE PE
as per-128-token-block matmuls: for block b,

    ctx[t] = A_t*hsum + (B_t-A_t)*hpre_b + sum_{s in block} M_b[t,s]*h[s]
    M_b[t,s] = (B_t-A_t) for s<t, -A_t for s=t, 0 for s>t

with A_t = 0.5/(S-1-t) (gated), B_t = 0.5/t (gated). One fp8 DoubleRow matmul
per (block, e-subblock): stationary group0 = the h block (natural token-major
layout), group1 = {hsum, hpre_b} rows in "double-fp8" (value + residual) with
zeros below; moving group0 = 64*M_b^T fp8, group1 = the 64*A/64*(B-A)
coefficient rows. hsum/hpre (cheap O(B*S*E) numpy sums) come from the host.
PSUM accumulates the whole context in f32, one Copy-activation casts it to fp8
ctx^T tiles, and the rest is the reference MLP as plain fp8 DoubleRow GEMMs
(host-prescaled weights, gelu via the scalar engine's scale/bias ports). No
scans, no transposes, no cross-core collectives.
"""

from contextlib import ExitStack

import numpy as np
import ml_dtypes

P = 128
B, S, E = 4, 4096, 1024
T = S // 2              # tokens per core
NB = T // P             # 16 token blocks per core
NC = 4                  # 512-token L1/L2/L3 chunks per core
CH = T // NC            # 512
F1, F2 = 1024, 512
NE, NF1, NF2 = E // P, F1 // P, F2 // P
NUM_TIMESTEPS = 1000
FP8_MAX = 240.0  # TRN float8e4 is IEEE e4m3 (inf/nan present): max finite 240

_COMPILED = None


def _build_nc(act="Gelu"):
    import concourse.mybir as mybir
    import concourse.tile as tile
    from concourse import bacc

    f32 = mybir.dt.float32
    fp8 = mybir.dt.float8e4
    AF = mybir.ActivationFunctionType
    ACT_FN = getattr(AF, act)
    DR = mybir.MatmulPerfMode.DoubleRow

    nc = bacc.Bacc("TRN2", target_bir_lowering=False, debug=False, num_devices=8)

    hN_d = nc.dram_tensor("hN", (P, NB, NE, P), fp8, kind="ExternalInput").ap()
    # 4-partition sliver {hsum, hsum_res, hpre_b, hpre_res} for hN group1
    aug_d = nc.dram_tensor("aug4", (4, NB, NE, P), fp8, kind="ExternalInput").ap()
    mvt_d = nc.dram_tensor("mvt", (P, NB, 2, P), fp8, kind="ExternalInput").ap()
    hT_d = nc.dram_tensor("hT", (P, NE, T), fp8, kind="ExternalInput").ap()
    w1_d = nc.dram_tensor("w1pk", (P, NF1, 8, 2, P), fp8, kind="ExternalInput").ap()
    w2_d = nc.dram_tensor("w2pk", (P, NF2, 4, 2, P), fp8, kind="ExternalInput").ap()
    w3_d = nc.dram_tensor("w3u", (P, 2, 2, 1), fp8, kind="ExternalInput").ap()
    bcs_d = nc.dram_tensor("bcs", (P, NF1 + NF2), f32, kind="ExternalInput").ap()
    # logits token-on-partition: out[p, b] = 32*logit(token b*128+p)
    out_d = nc.dram_tensor("out", (P, NB), f32, kind="ExternalOutput").ap()

    with tile.TileContext(nc) as tc, ExitStack() as ctx:
        const = ctx.enter_context(tc.tile_pool(name="const", bufs=1))
        big = ctx.enter_context(tc.tile_pool(name="big", bufs=1))
        psC = ctx.enter_context(tc.tile_pool(name="psC", bufs=4, space="PSUM"))
        psM = ctx.enter_context(tc.tile_pool(name="psM", bufs=3, space="PSUM"))
        psN = ctx.enter_context(tc.tile_pool(name="psN", bufs=1, space="PSUM"))

        # ---- SBUF tiles
        # hN group0 = the h block (token-partition); group1 = the aug rows
        # {hsum, hsum_res, hpre_b, hpre_res} on partitions 0-3 (memset + DMA
        # sliver), so each ctx matmul is ONE start+stop instruction — PSUM
        # zero-region semantics forbid concurrent multi-instruction groups
        # in one 2KB zone, and the scheduler interleaves column chains.
        hN_t = big.tile([P, NB, 2, NE, P], fp8, name="hN_t")
        mvt_t = const.tile([P, NB, 2, P], fp8, name="mvt_t")
        hT_t = big.tile([P, NE, T], fp8, name="hT_t")
        ctxT = big.tile([P, NE, T], fp8, name="ctxT")
        x1 = big.tile([P, NF1, T], fp8, name="x1")
        x2 = big.tile([P, NF2, T], fp8, name="x2")
        nf2 = big.tile([P, NB], f32, name="nf2")
        w1_t = big.tile([P, NF1, 8, 2, P], fp8, name="w1_t")
        w2_t = const.tile([P, NF2, 4, 2, P], fp8, name="w2_t")
        w3_t = const.tile([P, 2, 2, 1], fp8, name="w3_t")
        bcs = const.tile([P, NF1 + NF2], f32, name="bcs")

        # ---- DMA prologue. One HWDGE ring (SP): issue (625ns) and the
        # DMA-engine pool are both serial global resources in the cost model,
        # so what matters is copy COUNT and exact need-order of the byte
        # stream. Pool zeroes hN group1 ahead of the 4-partition aug slivers;
        # only the tiny w3 rides the Pool DGE.
        wq = nc.gpsimd

        def load_hN(b0, n):
            nc.gpsimd.memset(hN_t[:, b0:b0 + n, 1], 0.0)
            nc.sync.dma_start(hN_t[:, b0:b0 + n, 0], hN_d[:, b0:b0 + n])
            nc.sync.dma_start(hN_t[0:4, b0:b0 + n, 1], aug_d[:, b0:b0 + n])

        def load_w1(f0, n):  # n f-blocks, all 8 contraction layers: n*128KB
            nc.sync.dma_start(w1_t[:, f0:f0 + n], w1_d[:, f0:f0 + n])

        def load_hT(c):
            nc.sync.dma_start(
                hT_t[:, :, c * CH:(c + 1) * CH], hT_d[:, :, c * CH:(c + 1) * CH]
            )

        nc.sync.dma_start(mvt_t[:, 0:2], mvt_d[:, 0:2])
        load_hN(0, 1)
        load_hN(1, 2)
        load_hT(0)
        load_w1(0, 2)
        nc.sync.dma_start(bcs[:], bcs_d[:])
        load_hN(3, 3)
        load_w1(2, 2)
        nc.sync.dma_start(mvt_t[:, 2:8], mvt_d[:, 2:8])
        load_w1(4, 2)
        load_hN(6, 3)
        load_w1(6, 2)
        load_hT(1)
        nc.sync.dma_start(mvt_t[:, 8:], mvt_d[:, 8:])
        load_hN(9, 4)
        load_hT(2)
        nc.sync.dma_start(w2_t[:], w2_d[:])
        load_hT(3)
        load_hN(13, 3)
        wq.dma_start(w3_t[:], w3_d[:])

        # ---- context block: ONE start+stop DR matmul per e-sub (group0 =
        # h block x M^T, group1 = aug rows x coefficient rows), then a DVE
        # fp8 cast of the f32 PSUM accumulation per half block.
        def ctx_block(b):
            for h2 in range(2):
                psc = psC.tile([P, 4 * P], f32, tag="psC", name=f"psc_{b}_{h2}")
                for ai in range(4):
                    a = 4 * h2 + ai
                    nc.tensor.matmul(
                        psc[:, ai * P:(ai + 1) * P],
                        hN_t[:, b, :, a, :],
                        mvt_t[:, b, :, :],
                        start=True, stop=True,
                        perf_mode=DR,
                    )
                dst = ctxT[:, 4 * h2:4 * h2 + 4, b * P:(b + 1) * P]
                src = psc[:].rearrange("p (a t) -> p a t", a=4)
                # odd blocks put one half on ACT so the two copies overlap
                # (the Copy/Gelu act-table set loads only twice, not per switch)
                if b % 2 == 1 and h2 == 1:
                    nc.scalar.mul(dst, src, 1.0 / 16.0)
                else:
                    nc.vector.tensor_scalar_mul(dst, src, 1.0 / 16.0)

        ctx_iter = iter(range(NB))

        def emit_ctx(n):
            for _ in range(n):
                b = next(ctx_iter, None)
                if b is not None:
                    ctx_block(b)

        # ---- L1 (pre1 = [h, ctx] @ [W1a*32; W1b*8], gelu -> x1 fp8)
        def l1_pair(t0, tw, fg):
            tsl = slice(t0, t0 + tw)
            fs = (2 * fg, 2 * fg + 1)
            ps = [psM.tile([P, tw], f32, tag="psM", name=f"psP_{t0}_{f}") for f in fs]
            for e2 in range(8):
                if e2 < 4:
                    mov = hT_t[:, 2 * e2:2 * e2 + 2, tsl]
                else:
                    mov = ctxT[:, 2 * (e2 - 4):2 * (e2 - 4) + 2, tsl]
                for i, f in enumerate(fs):
                    nc.tensor.matmul(
                        ps[i], w1_t[:, f, e2], mov,
                        start=(e2 == 0), stop=(e2 == 7),
                        perf_mode=DR,
                    )
            for i, f in enumerate(fs):
                nc.scalar.activation(
                    x1[:, f, tsl], ps[i][:], ACT_FN,
                    bias=bcs[:, f:f + 1], scale=float(1.0 / 32.0),
                )

        def l2_one(t0, tw, f2):
            tsl = slice(t0, t0 + tw)
            psx = psM.tile([P, tw], f32, tag="psM", name=f"psX_{t0}_{f2}")
            for r in range(4):
                nc.tensor.matmul(
                    psx, w2_t[:, f2, r],
                    x1[:, 2 * r:2 * r + 2, tsl],
                    start=(r == 0), stop=(r == 3),
                    perf_mode=DR,
                )
            nc.scalar.activation(
                x2[:, f2, tsl], psx[:], ACT_FN,
                bias=bcs[:, NF1 + f2:NF1 + f2 + 1], scale=float(1.0 / 32.0),
            )

        # L3 with x2 as the DR stationary (dual-fp8 Ldweights forbids the
        # 1-wide w3 stationary) and w3 moving: one PSUM logit column per
        # (128-token block, r3 half) — every matmul is its own start+stop
        # group (zone-interleaving safe) — summed on the DVE.
        psn_t = psN.tile([P, 2, NB], f32, tag="psN", name="psn_t")

        def l3_one(t0, tw):
            c0 = t0 // P
            for tb in range(tw // P):
                for r3 in range(2):
                    nc.tensor.matmul(
                        psn_t[:, r3, c0 + tb:c0 + tb + 1],
                        x2[:, 2 * r3:2 * r3 + 2, t0 + tb * P:t0 + (tb + 1) * P],
                        w3_t[:, r3],
                        start=True, stop=True,
                        perf_mode=DR,
                    )
            # DVE may read at most one PSUM operand per instruction
            nc.vector.tensor_copy(nf2[:, c0:c0 + tw // P], psn_t[:, 0, c0:c0 + tw // P])
            nc.vector.tensor_add(
                nf2[:, c0:c0 + tw // P],
                nf2[:, c0:c0 + tw // P],
                psn_t[:, 1, c0:c0 + tw // P],
            )

        # ---- schedule: warm the PE with early ctx blocks, then pipeline
        # chunks; sprinkle remaining ctx blocks between units so the PE never
        # waits on a long ACT copy chain.
        emit_ctx(4)  # chunk 0's ctx layers read ctxT blocks 0-3
        chunks = [(0, 512), (512, 512), (1024, 512), (1536, 512)]
        for ci, (t0, tw) in enumerate(chunks):
            for fg in range(4):
                l1_pair(t0, tw, fg)
                emit_ctx(1)
            last = ci == len(chunks) - 1
            # split the last chunk's L2/L3 so the final gelu->L3->out cascade
            # drains over 256 tokens instead of 512
            for u0, uw in ([(t0, 256), (t0 + 256, 256)] if last else [(t0, tw)]):
                for f2 in range(NF2):
                    l2_one(u0, uw, f2)
                emit_ctx(1)
                l3_one(u0, uw)
        emit_ctx(NB)  # safety: flush any not yet emitted
        nc.sync.dma_start(out_d[:], nf2[:])

    nc.compile()
    return nc


def _get_compiled():
    global _COMPILED
    if _COMPILED is None:
        _COMPILED = _build_nc()
    return _COMPILED


def _f8(x):
    return np.clip(np.asarray(x, dtype=np.float32), -FP8_MAX, FP8_MAX).astype(
        ml_dtypes.float8_e4m3
    )


def _make_in_maps(inputs):
    h = np.ascontiguousarray(np.asarray(inputs["hidden_states"], dtype=np.float32))
    W1 = np.asarray(inputs["W1"], dtype=np.float32)
    W2 = np.asarray(inputs["W2"], dtype=np.float32)
    W3 = np.asarray(inputs["W3"], dtype=np.float32)
    b1 = np.asarray(inputs["b1"], dtype=np.float32)
    b2 = np.asarray(inputs["b2"], dtype=np.float32)

    i = np.arange(S, dtype=np.float64)
    A = np.where(i < S - 1, 0.5 / np.maximum(S - 1 - i, 1), 0.0).astype(np.float32)
    Bv = np.where(i > 0, 0.5 / np.maximum(i, 1), 0.0).astype(np.float32)
    BA = (Bv - A).astype(np.float32)

    # packed exactly in SBUF tile layout: w1pk[p, f, e2, g, x] = W1s[e2*256+g*128+p, f*128+x]
    w1s = _f8(np.vstack([W1[:E] * np.float32(32.0), W1[E:] * np.float32(8.0)]))
    w1_f8 = np.ascontiguousarray(
        w1s.reshape(8, 2, P, NF1, P).transpose(2, 3, 0, 1, 4)
    )
    w2_f8 = np.ascontiguousarray(
        _f8(W2 * np.float32(32.0)).reshape(4, 2, P, NF2, P).transpose(2, 3, 0, 1, 4)
    )
    # w3u[p, r3, g, 0] = 32*W3[r3*256 + g*128 + p]
    w3u = _f8((W3[:, 0] * np.float32(32.0)).reshape(2, 2, P).transpose(2, 0, 1)[..., None])
    b1c = np.ascontiguousarray(b1.reshape(NF1, P).T)
    b2c = np.ascontiguousarray(b2.reshape(NF2, P).T)

    sloc = np.arange(P)
    lower = sloc[:, None] < sloc[None, :]          # strictly below diagonal (s < t')
    eye = sloc[:, None] == sloc[None, :]

    in_maps = []
    for core in range(8):
        bi, half = divmod(core, 2)
        tg0 = half * T
        hs = h[bi, tg0:tg0 + T]                     # (T, E)
        Al = A[tg0:tg0 + T]
        BAl = BA[tg0:tg0 + T]

        hsum = h[bi].sum(0, dtype=np.float64).astype(np.float32)
        csum = np.concatenate(
            [np.zeros((1, E), np.float64), np.cumsum(h[bi], 0, dtype=np.float64)]
        )
        s1 = _f8(hsum)
        s2 = _f8(hsum - s1.astype(np.float32))

        # hN group0: [s, b, a, e'] = the h blocks. aug4: the 4-partition
        # group1 sliver {s1, s2, q1_b, q2_b} (device memsets the rest of
        # group1 to zero).
        hN = _f8(hs).reshape(NB, P, NE, P).transpose(1, 0, 2, 3)
        aug4 = np.zeros((4, NB, NE, P), dtype=ml_dtypes.float8_e4m3)
        aug4[0] = np.broadcast_to(s1.reshape(NE, P), (NB, NE, P))
        aug4[1] = np.broadcast_to(s2.reshape(NE, P), (NB, NE, P))
        hpre = csum[tg0 + np.arange(NB) * P].astype(np.float32)   # (NB, E)
        q1 = _f8(hpre)
        aug4[2] = q1.reshape(NB, NE, P)
        aug4[3] = _f8(hpre - q1.astype(np.float32)).reshape(NB, NE, P)

        # mvt: [s, b, g, t']  g0 = 64*M_b^T, g1 = coef rows 64*{A, A, BA, BA}
        # on partitions 0-3 (pairs the aug rows in hN group1), zeros below.
        Ab = Al.reshape(NB, P)
        BAb = BAl.reshape(NB, P)
        mvt = np.zeros((P, NB, 2, P), dtype=ml_dtypes.float8_e4m3)
        m = (
            lower[None, :, :] * (64.0 * BAb)[:, None, :]
            - eye[None, :, :] * (64.0 * Ab)[:, None, :]
        )                                            # (NB, s, t')
        mvt[:, :, 0] = _f8(m).transpose(1, 0, 2)
        mvt[0, :, 1] = _f8(64.0 * Ab)
        mvt[1, :, 1] = _f8(64.0 * Ab)
        mvt[2, :, 1] = _f8(64.0 * BAb)
        mvt[3, :, 1] = _f8(64.0 * BAb)

        hT = _f8(hs.T).reshape(NE, P, T).transpose(1, 0, 2)

        in_maps.append({
            "hN": np.ascontiguousarray(hN),
            "aug4": np.ascontiguousarray(aug4),
            "mvt": np.ascontiguousarray(mvt),
            "hT": np.ascontiguousarray(hT),
            "w1pk": w1_f8,
            "w2pk": w2_f8,
            "w3u": np.ascontiguousarray(w3u),
            "bcs": np.ascontiguousarray(np.hstack([b1c, b2c])),
        })
    return in_maps


def _finish(logits, inputs):
    b3 = np.asarray(inputs["b3"], dtype=np.float32)
    nfv = np.float32(1.0) / (np.float32(1.0) + np.exp(-(logits / np.float32(32.0) + b3[0])))
    gt = np.float32(np.asarray(inputs["global_timestep"]))
    mask = np.asarray(inputs["token_mask"])
    ad = gt * (np.float32(0.5) + nfv.astype(np.float32))
    ad = ad * (np.float32(1.0) + mask.astype(np.float32) * np.float32(0.3))
    ad = np.clip(ad, np.float32(0.0), np.float32(NUM_TIMESTEPS - 1))
    return ad.astype(np.int32)


def kernel(**inputs):
    from concourse import bass_utils

    nc = _get_compiled()
    in_maps = _make_in_maps(inputs)
    res = bass_utils.run_bass_kernel_spmd(nc, in_maps, core_ids=list(range(8)))
    logits = np.zeros((B, S), np.float32)
    for c in range(8):
        bi, half = divmod(c, 2)
        r = np.asarray(res.results[c]["out"])          # [P, NB], token b*128+p
        logits[bi, half * T:(half + 1) * T] = r.T.reshape(T)
    return _finish(logits, inputs)
